# revision 16
# baseline (speedup 1.0000x reference)
"""Trainium2 Bass kernel for BasicQuadRGBModel (quad-Bayer demosaic CNN).

Design (measured ~653 us HW exec vs ~4.0 ms fp32 baseline, rel err 7.4e-3):
  - data parallel over 8 cores, 2 images each; per-image 16 groups of
    32 rows (4 slabs x 8 rows); deep pipeline stages at group granularity
    (A=T layer-0, B=T-2 layer-1, C=T-4 layer-2+softmax, D-prep=T-5,
    D=T-7 chroma+outputs) so every PE block's inputs are finalized a full
    iteration before use; strip/halo copies are emitted in the slack.
  - conv = band-packed im2col matmuls: K=120 (10 xa x 12 ch), M=96
    (8 xo x 12 ch), N=512. Layer 0 packs all 3 ky in K=128 (bf16).
  - layers 1/2 run in fp8e4m3 (weights x16, rescaled at eviction/exp):
    one DoubleRow matmul contracts ky0+ky1 at half-rate-per-row via an
    overlapping [120, 2, 8, 64] moving AP, plus a second DoubleRow matmul
    pairing ky2 with a zero-weight slot (stride-0 j dim) at the same rate.
  - PSUM pair-tiles [96, 16, 64] span 2 banks (each matmul writes one
    bank) so evict/exp/EP process 1024 elems per instruction.
  - softmax: E = max(exp(x), 1) replaces relu+exp; EP fused as
    (psf max 0) * E in one scalar_tensor_tensor; 16-wide sum matmuls
    (wse16/wsep) + DVE reciprocal_approx_fast; no broadcast matmul.
  - chroma: one matmul per slab over a [104p] combined buffer (3 row-
    shifted copies of d = rb - g, plus the ky1 rows of r0) built by DMA.
  - f/w branch activations share one tile [120, 2, 34, 64] per layer so
    the xa-halo strip copies are 2 DMAs per layer per group; r0 group
    tiles carry a +-1 row halo so d is a single SBUF copy.
  - engine split: ACT = conv evicts + exp + half the chroma copies;
    DVE = E max, EP, reciprocal, g mul, d sub, memsets, other chroma copy;
    GpSimd = strip/combine DMA dispatch only; Sync = r0/g/d/out DMA.
  - float32r is deliberately avoided: its NEFFs wedge the NeuronCore
    (NRT_EXEC_UNIT_UNRECOVERABLE). bf16/fp8 run at the same PE rate.
  - host does layer-0 im2col (bf16) and the final 2x2 pixel-shuffle.
"""

import sys

sys.path.insert(0, "/opt/trn_rl_repo")

import ml_dtypes
import numpy as np

import concourse.bass as bass  # noqa: F401
import concourse.mybir as mybir
import concourse.tile as tile
from concourse import bacc
from concourse.bass_utils import run_bass_kernel_spmd

N_CORES = 8
B_PC = 2
H = W = 512
NW = 64
CH = 12
GS = 4
GROWS = GS * 8
NG_IMG = H // GROWS
NGROUP = B_PC * NG_IMG
F32 = mybir.dt.float32
BF16 = mybir.dt.bfloat16
F8 = mybir.dt.float8e4
BF16NP = ml_dtypes.bfloat16
F8NP = ml_dtypes.float8_e4m3
WSCALE = 16.0
SUMS96 = False


def _rbloc(xa, c):
    if xa == 0:
        return 16 + c
    if xa == 9:
        return 18 + c
    return (xa - 1) * 2 + c


def _rloc(xa, ci):
    if xa == 0:
        return 96 + ci
    if xa == 9:
        return 108 + ci
    return (xa - 1) * 12 + ci


def _r0loc(ky, ci, xa):
    if ky == 0:
        if ci == 0:
            return xa
        if ci == 3:
            return 10 + xa
        return 20 + _rbloc(xa, ci - 1)
    if ky == 1:
        if ci == 0:
            return 40 + xa
        if ci == 3:
            return 50 + xa
        return 64 + _rbloc(xa, ci - 1)
    if ci == 0:
        return 84 + xa
    if ci == 3:
        return 94 + xa
    return 104 + _rbloc(xa, ci - 1)


def build_r0(mosaic):
    B = mosaic.shape[0]
    mp = np.zeros((B, 4, H + 2, W + 2), BF16NP)
    mp[:, :, 1 : H + 1, 1 : W + 1] = mosaic.astype(BF16NP)
    r0 = np.zeros((B, 128, H, NW), BF16NP)
    for ky in range(3):
        for ci in range(4):
            for xa in range(10):
                r0[:, _r0loc(ky, ci, xa)] = mp[:, ci, ky : ky + H, xa : xa + 8 * NW : 8]
    return r0


def build_w_l0(wt):
    W_ = np.zeros((128, 96), np.float32)
    for ky in range(3):
        for ci in range(4):
            for xa in range(10):
                for xo in range(8):
                    kx = xa - xo
                    if 0 <= kx <= 2:
                        for co in range(CH):
                            W_[_r0loc(ky, ci, xa), xo * 12 + co] = wt[co, ci, ky, kx]
    return W_


def build_w_int(wt):
    W_ = np.zeros((3, 120, 96), np.float32)
    for ky in range(3):
        for xa in range(10):
            for xo in range(8):
                kx = xa - xo
                if 0 <= kx <= 2:
                    k = _rloc(xa, 0)
                    W_[ky, k : k + 12, xo * 12 : xo * 12 + 12] = wt[:, :, ky, kx].T
    return W_


def build_w_sums16():
    wse16 = np.zeros((96, 16), np.float32)
    wsep = np.zeros((96, 16), np.float32)
    for xo in range(8):
        for co in range(CH):
            wse16[xo * 12 + co, 2 * xo] = 1.0
            wse16[xo * 12 + co, 2 * xo + 1] = 1.0
            wsep[xo * 12 + co, xo * 2 + (co >= 6)] = 1.0 / WSCALE
    return wse16, wsep


def build_w_chroma_rt(cw0):
    # chroma = conv(rb) - conv(g) + green_add;  this is the conv(rb) part
    # plus green_add's m0/m3 terms, read straight from the r0 tile (all 3
    # ky blocks present as row-shifted planes).
    W = np.zeros((128, 48), np.float32)
    for ky in range(3):
        for xa in range(10):
            for xo in range(8):
                kx = xa - xo
                if 0 <= kx <= 2:
                    for co in range(6):
                        for d in range(2):
                            W[_r0loc(ky, d + 1, xa), xo * 6 + co] += cw0[co, d, ky, kx]
    # green_add m parts: [m0, _, m3, m0, _, m3] at center tap
    for xo in range(8):
        xa = xo + 1
        W[_r0loc(1, 0, xa), xo * 6 + 0] += 1.0
        W[_r0loc(1, 0, xa), xo * 6 + 3] += 1.0
        W[_r0loc(1, 3, xa), xo * 6 + 2] += 1.0
        W[_r0loc(1, 3, xa), xo * 6 + 5] += 1.0
    return W


def build_w_chroma_g(cw0):
    # -conv(g) over 3 ky-shifted copies of the g plane, plus green_add's
    # g1/g0 terms at the center tap.
    W = np.zeros((60, 48), np.float32)
    for ky in range(3):
        for xa in range(10):
            for xo in range(8):
                kx = xa - xo
                if 0 <= kx <= 2:
                    for co in range(6):
                        for gh in range(2):
                            W[20 * ky + _rbloc(xa, gh), xo * 6 + co] -= cw0[
                                co, gh, ky, kx
                            ]
    for xo in range(8):
        xa = xo + 1
        W[20 * 1 + _rbloc(xa, 1), xo * 6 + 1] += 1.0
        W[20 * 1 + _rbloc(xa, 0), xo * 6 + 4] += 1.0
    return W


def assemble_output(mosaic, cp_dev, g_dev):
    B = mosaic.shape[0]
    cp = (
        cp_dev.astype(np.float32)
        .reshape(B, 8, 6, H, NW)
        .transpose(0, 2, 3, 4, 1)
        .reshape(B, 6, H, W)
    )
    g = (
        g_dev.astype(np.float32)
        .reshape(B, 8, 2, H, NW)
        .transpose(0, 2, 3, 4, 1)
        .reshape(B, 2, H, W)
    )
    m = mosaic
    out = np.empty((B, 3, 2 * H, 2 * W), np.float32)
    out[:, 0, 0::2, 0::2] = cp[:, 0]
    out[:, 0, 0::2, 1::2] = m[:, 1]
    out[:, 0, 1::2, 0::2] = cp[:, 1]
    out[:, 0, 1::2, 1::2] = cp[:, 2]
    out[:, 1, 0::2, 0::2] = m[:, 0]
    out[:, 1, 0::2, 1::2] = g[:, 0]
    out[:, 1, 1::2, 0::2] = g[:, 1]
    out[:, 1, 1::2, 1::2] = m[:, 3]
    out[:, 2, 0::2, 0::2] = cp[:, 3]
    out[:, 2, 0::2, 1::2] = cp[:, 4]
    out[:, 2, 1::2, 0::2] = m[:, 2]
    out[:, 2, 1::2, 1::2] = cp[:, 5]
    return out


# column offsets inside the packed [128, _WCOLS] bf16 stationary tensor
_WOFF = {"wf0": 0, "ww0": 96,
         "wse16": 192, "wsep": 208, "wch_rt": 224, "wch_g": 272}
_WCOLS = 320
# fp8 stationary tensor [128, 16, 96]: blocks 2i,2i+1 = conv i ky0/ky1 (DoubleRow
# pair), blocks 8+2i,8+2i+1 = conv i (ky2, zeros) DR pair; conv order
# (wf1, ww1, wf2, ww2); scaled by WSCALE
_W8ORDER = ("wf1", "ww1", "wf2", "ww2")


def pack_stationaries(st):
    wp = np.zeros((128, _WCOLS), np.float32)
    wp[:, 0:96] = st["wf0"]
    wp[:, 96:192] = st["ww0"]
    wp[0:96, 192:208] = st["wse16"]
    wp[0:96, 208:224] = st["wsep"]
    wp[:, 224:272] = st["wch_rt"]
    wp[0:60, 272:320] = st["wch_g"]
    w8 = np.zeros((128, 16, 96), np.float32)
    for i, nm in enumerate(_W8ORDER):
        w8[0:120, 2 * i, :] = st[nm][0] * WSCALE
        w8[0:120, 2 * i + 1, :] = st[nm][1] * WSCALE
        w8[0:120, 8 + 2 * i, :] = st[nm][2] * WSCALE
    return wp, w8


def build_program():
    from contextlib import ExitStack

    nc = bacc.Bacc(
        "TRN2", target_bir_lowering=False, debug=False, num_devices=N_CORES
    )
    r0 = nc.declare_dram_parameter("r0", [B_PC, 128, H, NW], BF16, isOutput=False)
    wpack = nc.declare_dram_parameter("wpack", [128, _WCOLS], BF16, isOutput=False)
    wpack8 = nc.declare_dram_parameter("wpack8", [128, 16, 96], F8, isOutput=False)
    out_cp = nc.declare_dram_parameter("out_cp", [B_PC, 48, H, NW], BF16, isOutput=True)
    out_g = nc.declare_dram_parameter("out_g", [B_PC, 16, H, NW], BF16, isOutput=True)

    Relu = mybir.ActivationFunctionType.Relu
    Exp = mybir.ActivationFunctionType.Exp
    Copy = mybir.ActivationFunctionType.Copy

    with tile.TileContext(nc) as tc, ExitStack() as ctx:
        const = ctx.enter_context(tc.tile_pool(name="const", bufs=1))
        r0pool = ctx.enter_context(tc.tile_pool(name="r0pool", bufs=11))
        p_r1 = ctx.enter_context(tc.tile_pool(name="r1", bufs=5))
        p_r2 = ctx.enter_context(tc.tile_pool(name="r2", bufs=6))
        p_grb = ctx.enter_context(tc.tile_pool(name="grb", bufs=7))
        p_gsh = ctx.enter_context(tc.tile_pool(name="gsh", bufs=4))
        p_act = ctx.enter_context(tc.tile_pool(name="acts", bufs=4))
        p_rcp = ctx.enter_context(tc.tile_pool(name="rcp", bufs=4))
        p_stg = ctx.enter_context(tc.tile_pool(name="stg", bufs=3))
        ps_mm = ctx.enter_context(tc.tile_pool(name="psmm", bufs=3, space="PSUM"))
        ps_sm = ctx.enter_context(tc.tile_pool(name="pssm", bufs=2, space="PSUM"))

        WC = const.tile([128, _WCOLS], BF16, tag="wpack_sb", name="wpack_sb")
        nc.sync.dma_start(out=WC[:], in_=wpack[:])
        WC8 = const.tile([128, 16, 96], F8, tag="wpack8_sb", name="wpack8_sb")
        nc.sync.dma_start(out=WC8[:], in_=wpack8[:])
        sb = {
            "wf0": WC[:, 0:96],
            "ww0": WC[:, 96:192],
            "wse16": WC[0:96, 192:208],
            "wsep": WC[0:96, 208:224],
            "wch_rt": WC[:, 224:272],
            "wch_g": WC[0:60, 272:320],
        }

        def w8dr(i):
            return WC8[0:120, 2 * i : 2 * i + 2, :]

        def w8ky2(i):
            return WC8[0:120, 8 + 2 * i : 8 + 2 * i + 2, :]

        def dr_moving(t, br, s, ky0=0, jstep=NW):
            # [120, 2, 8, NW] view of rows 8s+ky0..: j dim strides jstep elems
            mv = t[:, br : br + 1, 8 * s + ky0 : 8 * s + ky0 + 8, :]
            apv = mv.ap
            apv[1] = (jstep, 2)
            mv.ap = apv
            return mv

        r0s, r1, r2, grb, gshs, stash = {}, {}, {}, {}, {}, {}

        def load_r0(g):
            if g in r0s or not (0 <= g < NGROUP):
                return r0s.get(g)
            img, gi = divmod(g, NG_IMG)
            y0 = gi * GROWS
            rt = r0pool.tile([128, GROWS + 2, NW], BF16, name="rt")
            r0s[g] = rt
            if gi == 0:
                nc.vector.memset(rt[:, 0:1, :], 0.0)
                nc.sync.dma_start(
                    out=rt[:, 1 : GROWS + 2, :], in_=r0[img, :, 0 : GROWS + 1, :]
                )
            elif gi == NG_IMG - 1:
                nc.vector.memset(rt[:, GROWS + 1 : GROWS + 2, :], 0.0)
                nc.sync.dma_start(
                    out=rt[:, 0 : GROWS + 1, :],
                    in_=r0[img, :, y0 - 1 : y0 + GROWS, :],
                )
            else:
                nc.sync.dma_start(
                    out=rt[:], in_=r0[img, :, y0 - 1 : y0 + GROWS + 1, :]
                )
            return rt

        def get_rbuf(pool, dct, g):
            if g in dct or not (0 <= g < NGROUP):
                return dct.get(g)
            t = pool.tile([120, 2, GROWS + 2, NW], F8)
            dct[g] = t
            gi = g % NG_IMG
            if gi == 0:
                nc.gpsimd.memset(t[0:96, :, 0:1, :], 0.0)
            if gi == NG_IMG - 1:
                nc.gpsimd.memset(t[0:96, :, GROWS + 1 : GROWS + 2, :], 0.0)
            nc.gpsimd.memset(t[96:120, :, :, 0:1], 0.0)
            nc.gpsimd.memset(t[96:120, :, :, NW - 1 : NW], 0.0)
            return t

        def get_grb(g):
            if g in grb or not (0 <= g < NGROUP):
                return grb.get(g)
            t = p_grb.tile([20, GROWS + 2, NW], BF16, name="g")
            grb[g] = t
            gi = g % NG_IMG
            if gi == 0:
                nc.gpsimd.memset(t[:, 0:1, :], 0.0)
            if gi == NG_IMG - 1:
                nc.gpsimd.memset(t[:, GROWS + 1 : GROWS + 2, :], 0.0)
            nc.gpsimd.memset(t[:, :, 0:1], 0.0)
            nc.gpsimd.memset(t[:, :, NW - 1 : NW], 0.0)
            return t

        Max = mybir.AluOpType.max
        Mult = mybir.AluOpType.mult

        def evict_pair(ps, dct, br, g, gi, t, scale=1.0):
            # ps holds slabs 2t and 2t+1 in one [96, 16, NW] double tile
            nc.scalar.activation(
                out=dct[g][0:96, br, 16 * t + 1 : 16 * t + 17, :],
                in_=ps[:],
                func=Relu,
                scale=scale,
            )
            # boundary-row halo copies into neighbor tiles go to DVE
            if t == 0 and gi > 0:
                nc.vector.tensor_scalar(
                    out=dct[g - 1][0:96, br, GROWS + 1 : GROWS + 2, :],
                    in0=ps[:, 0:1, :],
                    scalar1=0.0,
                    scalar2=scale,
                    op0=Max,
                    op1=Mult,
                )
            if t == 1 and gi < NG_IMG - 1:
                nc.vector.tensor_scalar(
                    out=dct[g + 1][0:96, br, 0:1, :],
                    in0=ps[:, 15:16, :],
                    scalar1=0.0,
                    scalar2=scale,
                    op0=Max,
                    op1=Mult,
                )

        def strips(t):
            nc.gpsimd.dma_start(
                out=t[96:108, :, :, 1:NW], in_=t[84:96, :, :, 0 : NW - 1]
            )
            nc.gpsimd.dma_start(
                out=t[108:120, :, :, 0 : NW - 1], in_=t[0:12, :, :, 1:NW]
            )

        for T in range(NGROUP + 8):
            load_r0(T)
            load_r0(T + 1)
            load_r0(T + 2)
            # ---- stage A: group a = T; layer-0 convs ----
            a = T
            if 0 <= a < NGROUP:
                img, gi = divmod(a, NG_IMG)
                rt = r0s[a]
                get_rbuf(p_r1, r1, a)
                get_rbuf(p_r1, r1, a + 1)
                for br, nm in ((0, "wf0"), (1, "ww0")):
                    for t in range(2):
                        ps = ps_mm.tile([96, 16, NW], F32, tag="mm96", name="ps0")
                        for u in range(2):
                            s = 2 * t + u
                            nc.tensor.matmul(
                                ps[:, 8 * u : 8 * u + 8, :],
                                sb[nm],
                                rt[:, 8 * s + 1 : 8 * s + 9, :],
                                start=True,
                                stop=True,
                            )
                        evict_pair(ps, r1, br, a, gi, t)
            if 0 <= T - 1 < NGROUP:
                strips(r1[T - 1])

            # ---- stage B: group b = T - 2; layer-1 convs ----
            b = T - 2
            if 0 <= b < NGROUP:
                img, gi = divmod(b, NG_IMG)
                get_rbuf(p_r2, r2, b)
                get_rbuf(p_r2, r2, b + 1)
                for br, wi in ((0, 0), (1, 1)):
                    pss = [
                        ps_mm.tile([96, 16, NW], F32, tag="mm96", name="ps1")
                        for _ in range(2)
                    ]
                    for s in range(GS):
                        nc.tensor.matmul(
                            pss[s // 2][:, 8 * (s % 2) : 8 * (s % 2) + 8, :],
                            w8dr(wi),
                            dr_moving(r1[b], br, s),
                            start=True,
                            stop=False,
                            perf_mode=mybir.MatmulPerfMode.DoubleRow,
                        )
                    for s in range(GS):
                        nc.tensor.matmul(
                            pss[s // 2][:, 8 * (s % 2) : 8 * (s % 2) + 8, :],
                            w8ky2(wi),
                            dr_moving(r1[b], br, s, ky0=2, jstep=0),
                            start=False,
                            stop=True,
                            perf_mode=mybir.MatmulPerfMode.DoubleRow,
                        )
                    for t in range(2):
                        evict_pair(pss[t], r2, br, b, gi, t, scale=1.0 / WSCALE)
            if 0 <= T - 3 < NGROUP:
                strips(r2[T - 3])

            # ---- stage C: group c = T - 4; layer-2 convs + softmax sums ----
            c = T - 4
            if 0 <= c < NGROUP:
                img, gi = divmod(c, NG_IMG)
                gt = get_grb(c)
                get_grb(c + 1)
                Es, EPs = [], []
                pss = [
                    ps_mm.tile([96, 16, NW], F32, tag="mm96", name="psw2")
                    for _ in range(2)
                ]
                for s in range(GS):
                    nc.tensor.matmul(
                        pss[s // 2][:, 8 * (s % 2) : 8 * (s % 2) + 8, :],
                        w8dr(3),
                        dr_moving(r2[c], 1, s),
                        start=True,
                        stop=False,
                        perf_mode=mybir.MatmulPerfMode.DoubleRow,
                    )
                for s in range(GS):
                    nc.tensor.matmul(
                        pss[s // 2][:, 8 * (s % 2) : 8 * (s % 2) + 8, :],
                        w8ky2(3),
                        dr_moving(r2[c], 1, s, ky0=2, jstep=0),
                        start=False,
                        stop=True,
                        perf_mode=mybir.MatmulPerfMode.DoubleRow,
                    )
                for t in range(2):
                    E0 = p_act.tile([96, 16, NW], BF16, tag="E0", name="E0")
                    nc.scalar.activation(
                        out=E0[:], in_=pss[t][:], func=Exp, scale=1.0 / WSCALE
                    )
                    E = p_act.tile([96, 16, NW], BF16, tag="E", name="E")
                    nc.vector.tensor_scalar_max(E[:], E0[:], 1.0)
                    Es.append(E)
                pss = [
                    ps_mm.tile([96, 16, NW], F32, tag="mm96", name="psf2")
                    for _ in range(2)
                ]
                for s in range(GS):
                    nc.tensor.matmul(
                        pss[s // 2][:, 8 * (s % 2) : 8 * (s % 2) + 8, :],
                        w8dr(2),
                        dr_moving(r2[c], 0, s),
                        start=True,
                        stop=False,
                        perf_mode=mybir.MatmulPerfMode.DoubleRow,
                    )
                for s in range(GS):
                    nc.tensor.matmul(
                        pss[s // 2][:, 8 * (s % 2) : 8 * (s % 2) + 8, :],
                        w8ky2(2),
                        dr_moving(r2[c], 0, s, ky0=2, jstep=0),
                        start=False,
                        stop=True,
                        perf_mode=mybir.MatmulPerfMode.DoubleRow,
                    )
                for t in range(2):
                    EP = p_act.tile([96, 16, NW], BF16, tag="EP", name="EP")
                    nc.vector.scalar_tensor_tensor(
                        out=EP[:],
                        in0=pss[t][:],
                        scalar=0.0,
                        in1=Es[t][:],
                        op0=mybir.AluOpType.max,
                        op1=mybir.AluOpType.mult,
                    )
                    EPs.append(EP)
                rcps = []
                for s in range(GS):
                    # sums land at psum partitions 96-111 (PE col group 3,
                    # disjoint from the conv matmuls' columns 0-95)
                    if SUMS96:
                        pset = ps_sm.tile([112, 8, NW], F32, tag="sm", name="pse")
                        pse = pset[96:112]
                        tp = (0, 96)
                    else:
                        pse = ps_sm.tile([16, 8, NW], F32, tag="sm", name="pse")[:]
                        tp = None
                    nc.tensor.matmul(
                        pse,
                        sb["wse16"],
                        Es[s // 2][:, 8 * (s % 2) : 8 * (s % 2) + 8, :],
                        start=True,
                        stop=True,
                        tile_position=tp,
                    )
                    rcp = p_rcp.tile([16, 8, NW], F32, tag="rcp", name="rcp")
                    nc.vector.reciprocal_approx_fast(out=rcp[:], in_=pse)
                    rcps.append(rcp)
            # ---- stage D: group gD = T - 7; chroma conv + staging ----
            gD = T - 7
            if 0 <= gD < NGROUP:
                imgD, giD = divmod(gD, NG_IMG)
                gtD = grb[gD]
                rtD = r0s[gD]
                gshD = gshs[gD]
                stgt = p_stg.tile([48, GROWS, NW], BF16, name="stg")
                for t in range(2):
                    pc = ps_mm.tile([96, 16, NW], F32, tag="mm96", name="pc")
                    for u in range(2):
                        s = 2 * t + u
                        # chroma = conv(rb) + green_add m-parts (from r0)
                        #        - conv(g) + green_add g-parts (from gsh)
                        nc.tensor.matmul(
                            pc[0:48, 8 * u : 8 * u + 8, :],
                            sb["wch_rt"],
                            rtD[:, 8 * s + 1 : 8 * s + 9, :],
                            start=True,
                            stop=False,
                        )
                        nc.tensor.matmul(
                            pc[0:48, 8 * u : 8 * u + 8, :],
                            sb["wch_g"],
                            gshD[:, 8 * s : 8 * s + 8, :],
                            start=False,
                            stop=True,
                        )
                    if t == 0:
                        nc.scalar.activation(
                            out=stgt[:, 0:16, :], in_=pc[0:48, :, :], func=Copy
                        )
                    else:
                        nc.vector.tensor_copy(
                            out=stgt[:, 16:32, :], in_=pc[0:48, :, :]
                        )
                stash[gD] = (imgD, giD * GROWS, stgt, gtD)


            # ---- stage C part 2: psep sums + green multiply ----
            if 0 <= c < NGROUP:
                for s in range(GS):
                    if SUMS96:
                        psept = ps_sm.tile([112, 8, NW], F32, tag="sm", name="psep")
                        psep = psept[96:112]
                        tp = (0, 96)
                    else:
                        psep = ps_sm.tile([16, 8, NW], F32, tag="sm", name="psep")[:]
                        tp = None
                    nc.tensor.matmul(
                        psep,
                        sb["wsep"],
                        EPs[s // 2][:, 8 * (s % 2) : 8 * (s % 2) + 8, :],
                        start=True,
                        stop=True,
                        tile_position=tp,
                    )
                    nc.vector.tensor_mul(
                        gt[0:16, 8 * s + 1 : 8 * s + 9, :], psep, rcps[s][:]
                    )
                    if s == 0 and gi > 0:
                        nc.vector.tensor_mul(
                            grb[c - 1][0:16, GROWS + 1 : GROWS + 2, :],
                            psep[:, 0:1, :],
                            rcps[s][:, 0:1, :],
                        )
                    if s == GS - 1 and gi < NG_IMG - 1:
                        nc.vector.tensor_mul(
                            grb[c + 1][0:16, 0:1, :],
                            psep[:, 7:8, :],
                            rcps[s][:, 7:8, :],
                        )

            # ---- D-prep: group p = T - 5; g x-strips + 3 ky-shifted g copies ----
            p = T - 5
            if 0 <= p < NGROUP:
                gt = grb[p]
                nc.sync.dma_start(out=gt[16:18, :, 1:NW], in_=gt[14:16, :, 0 : NW - 1])
                nc.sync.dma_start(out=gt[18:20, :, 0 : NW - 1], in_=gt[0:2, :, 1:NW])
                gs = p_gsh.tile([60, GROWS, NW], BF16, name="gsh")
                gshs[p] = gs
                for k in range(3):
                    nc.gpsimd.dma_start(
                        out=gs[20 * k : 20 * (k + 1), :, :], in_=gt[:, k : k + GROWS, :]
                    )

            if 0 <= T - 8 < NGROUP:
                imgD, y0D, stgt, gtD = stash.pop(T - 8)
                nc.sync.dma_start(
                    out=out_cp[imgD, :, y0D : y0D + GROWS, :], in_=stgt[:]
                )
                nc.sync.dma_start(
                    out=out_g[imgD, :, y0D : y0D + GROWS, :],
                    in_=gtD[0:16, 1 : GROWS + 1, :],
                )
                r0s.pop(T - 7, None)
                r1.pop(T - 3, None)
                r2.pop(T - 5, None)
                grb.pop(T - 8, None)
                gshs.pop(T - 8, None)

    nc.compile()
    return nc


_CACHE = {}


def kernel(mosaic, fw0, fw1, fw2, ww0, ww1, ww2, cw0, _trace=False):
    mosaic = np.asarray(mosaic, np.float32)
    r0_all = build_r0(mosaic)

    stat = {
        "wf0": build_w_l0(np.asarray(fw0, np.float32)),
        "ww0": build_w_l0(np.asarray(ww0, np.float32)),
        "wf1": build_w_int(np.asarray(fw1, np.float32)),
        "wf2": build_w_int(np.asarray(fw2, np.float32)),
        "ww1": build_w_int(np.asarray(ww1, np.float32)),
        "ww2": build_w_int(np.asarray(ww2, np.float32)),
    }
    stat["wse16"], stat["wsep"] = build_w_sums16()
    cw0f = np.asarray(cw0, np.float32)
    stat["wch_rt"] = build_w_chroma_rt(cw0f)
    stat["wch_g"] = build_w_chroma_g(cw0f)
    wp, w8 = pack_stationaries(stat)
    wpack = wp.astype(BF16NP)
    wpack8 = w8.astype(F8NP)

    if "nc" not in _CACHE:
        _CACHE["nc"] = build_program()
    nc = _CACHE["nc"]

    in_maps = []
    for c in range(N_CORES):
        in_maps.append(
            {"r0": np.ascontiguousarray(r0_all[c * B_PC : (c + 1) * B_PC]),
             "wpack": wpack, "wpack8": wpack8}
        )

    res = run_bass_kernel_spmd(nc, in_maps, list(range(N_CORES)), trace=_trace)
    outs = []
    for c in range(N_CORES):
        outs.append(
            assemble_output(
                mosaic[c * B_PC : (c + 1) * B_PC],
                res.results[c]["out_cp"],
                res.results[c]["out_g"],
            )
        )
    full = np.concatenate(outs, axis=0)
    if _trace:
        return full, res
    return full



# revision 22
# speedup vs baseline: 1.0192x; 1.0192x over previous
"""Trainium2 Bass kernel for BasicQuadRGBModel (quad-Bayer demosaic CNN).

Design (measured ~653 us HW exec vs ~4.0 ms fp32 baseline, rel err 7.4e-3):
  - data parallel over 8 cores, 2 images each; per-image 16 groups of
    32 rows (4 slabs x 8 rows); deep pipeline stages at group granularity
    (A=T layer-0, B=T-2 layer-1, C=T-4 layer-2+softmax, D-prep=T-5,
    D=T-7 chroma+outputs) so every PE block's inputs are finalized a full
    iteration before use; strip/halo copies are emitted in the slack.
  - conv = band-packed im2col matmuls: K=120 (10 xa x 12 ch), M=96
    (8 xo x 12 ch), N=512. Layer 0 packs all 3 ky in K=128 (bf16).
  - layers 1/2 run in fp8e4m3 (weights x16, rescaled at eviction/exp):
    one DoubleRow matmul contracts ky0+ky1 at half-rate-per-row via an
    overlapping [120, 2, 8, 64] moving AP, plus a second DoubleRow matmul
    pairing ky2 with a zero-weight slot (stride-0 j dim) at the same rate.
  - PSUM pair-tiles [96, 16, 64] span 2 banks (each matmul writes one
    bank) so evict/exp/EP process 1024 elems per instruction.
  - softmax: E = max(exp(x), 1) replaces relu+exp; EP fused as
    (psf max 0) * E in one scalar_tensor_tensor; 16-wide sum matmuls
    (wse16/wsep) + DVE reciprocal_approx_fast; no broadcast matmul.
  - chroma: one matmul per slab over a [104p] combined buffer (3 row-
    shifted copies of d = rb - g, plus the ky1 rows of r0) built by DMA.
  - f/w branch activations share one tile [120, 2, 34, 64] per layer so
    the xa-halo strip copies are 2 DMAs per layer per group; r0 group
    tiles carry a +-1 row halo so d is a single SBUF copy.
  - engine split: ACT = conv evicts + exp + half the chroma copies;
    DVE = E max, EP, reciprocal, g mul, d sub, memsets, other chroma copy;
    GpSimd = strip/combine DMA dispatch only; Sync = r0/g/d/out DMA.
  - float32r is deliberately avoided: its NEFFs wedge the NeuronCore
    (NRT_EXEC_UNIT_UNRECOVERABLE). bf16/fp8 run at the same PE rate.
  - host does layer-0 im2col (bf16) and the final 2x2 pixel-shuffle.
"""

import sys

sys.path.insert(0, "/opt/trn_rl_repo")

import ml_dtypes
import numpy as np

import concourse.bass as bass  # noqa: F401
import concourse.mybir as mybir
import concourse.tile as tile
from concourse import bacc
from concourse.bass_utils import run_bass_kernel_spmd

N_CORES = 8
B_PC = 2
H = W = 512
NW = 64
CH = 12
GS = 4
GROWS = GS * 8
NG_IMG = H // GROWS
NGROUP = B_PC * NG_IMG
F32 = mybir.dt.float32
BF16 = mybir.dt.bfloat16
F8 = mybir.dt.float8e4
BF16NP = ml_dtypes.bfloat16
F8NP = ml_dtypes.float8_e4m3
WSCALE = 16.0
SUMS96 = False


def _rbloc(xa, c):
    if xa == 0:
        return 16 + c
    if xa == 9:
        return 18 + c
    return (xa - 1) * 2 + c


def _rloc(xa, ci):
    if xa == 0:
        return 96 + ci
    if xa == 9:
        return 108 + ci
    return (xa - 1) * 12 + ci


def _r0loc(ky, ci, xa):
    if ky == 0:
        if ci == 0:
            return xa
        if ci == 3:
            return 10 + xa
        return 20 + _rbloc(xa, ci - 1)
    if ky == 1:
        if ci == 0:
            return 40 + xa
        if ci == 3:
            return 50 + xa
        return 64 + _rbloc(xa, ci - 1)
    if ci == 0:
        return 84 + xa
    if ci == 3:
        return 94 + xa
    return 104 + _rbloc(xa, ci - 1)


def build_r0(mosaic):
    B = mosaic.shape[0]
    mp = np.zeros((B, 4, H + 2, W + 2), BF16NP)
    mp[:, :, 1 : H + 1, 1 : W + 1] = mosaic.astype(BF16NP)
    r0 = np.zeros((B, 128, H, NW), BF16NP)
    for ky in range(3):
        for ci in range(4):
            for xa in range(10):
                r0[:, _r0loc(ky, ci, xa)] = mp[:, ci, ky : ky + H, xa : xa + 8 * NW : 8]
    return r0


def build_w_l0(wt):
    W_ = np.zeros((128, 96), np.float32)
    for ky in range(3):
        for ci in range(4):
            for xa in range(10):
                for xo in range(8):
                    kx = xa - xo
                    if 0 <= kx <= 2:
                        for co in range(CH):
                            W_[_r0loc(ky, ci, xa), xo * 12 + co] = wt[co, ci, ky, kx]
    return W_


def build_w_int(wt):
    W_ = np.zeros((3, 120, 96), np.float32)
    for ky in range(3):
        for xa in range(10):
            for xo in range(8):
                kx = xa - xo
                if 0 <= kx <= 2:
                    k = _rloc(xa, 0)
                    W_[ky, k : k + 12, xo * 12 : xo * 12 + 12] = wt[:, :, ky, kx].T
    return W_


def build_w_sums16():
    wse16 = np.zeros((96, 16), np.float32)
    wsep = np.zeros((96, 16), np.float32)
    for xo in range(8):
        for co in range(CH):
            wse16[xo * 12 + co, 2 * xo] = 1.0
            wse16[xo * 12 + co, 2 * xo + 1] = 1.0
            wsep[xo * 12 + co, xo * 2 + (co >= 6)] = 1.0 / WSCALE
    return wse16, wsep


def build_w_chroma_rt(cw0):
    # chroma = conv(rb) - conv(g) + green_add;  this is the conv(rb) part
    # plus green_add's m0/m3 terms, read straight from the r0 tile (all 3
    # ky blocks present as row-shifted planes).
    W = np.zeros((128, 48), np.float32)
    for ky in range(3):
        for xa in range(10):
            for xo in range(8):
                kx = xa - xo
                if 0 <= kx <= 2:
                    for co in range(6):
                        for d in range(2):
                            W[_r0loc(ky, d + 1, xa), xo * 6 + co] += cw0[co, d, ky, kx]
    # green_add m parts: [m0, _, m3, m0, _, m3] at center tap
    for xo in range(8):
        xa = xo + 1
        W[_r0loc(1, 0, xa), xo * 6 + 0] += 1.0
        W[_r0loc(1, 0, xa), xo * 6 + 3] += 1.0
        W[_r0loc(1, 3, xa), xo * 6 + 2] += 1.0
        W[_r0loc(1, 3, xa), xo * 6 + 5] += 1.0
    return W


def build_w_chroma_g(cw0):
    # -conv(g) over 3 ky-shifted copies of the g plane, plus green_add's
    # g1/g0 terms at the center tap.
    W = np.zeros((60, 48), np.float32)
    for ky in range(3):
        for xa in range(10):
            for xo in range(8):
                kx = xa - xo
                if 0 <= kx <= 2:
                    for co in range(6):
                        for gh in range(2):
                            W[20 * ky + _rbloc(xa, gh), xo * 6 + co] -= cw0[
                                co, gh, ky, kx
                            ]
    for xo in range(8):
        xa = xo + 1
        W[20 * 1 + _rbloc(xa, 1), xo * 6 + 1] += 1.0
        W[20 * 1 + _rbloc(xa, 0), xo * 6 + 4] += 1.0
    return W


def assemble_output(mosaic, cp_dev, g_dev):
    B = mosaic.shape[0]
    cp = (
        cp_dev.astype(np.float32)
        .reshape(B, 8, 6, H, NW)
        .transpose(0, 2, 3, 4, 1)
        .reshape(B, 6, H, W)
    )
    g = (
        g_dev.astype(np.float32)
        .reshape(B, 8, 2, H, NW)
        .transpose(0, 2, 3, 4, 1)
        .reshape(B, 2, H, W)
    )
    m = mosaic
    out = np.empty((B, 3, 2 * H, 2 * W), np.float32)
    out[:, 0, 0::2, 0::2] = cp[:, 0]
    out[:, 0, 0::2, 1::2] = m[:, 1]
    out[:, 0, 1::2, 0::2] = cp[:, 1]
    out[:, 0, 1::2, 1::2] = cp[:, 2]
    out[:, 1, 0::2, 0::2] = m[:, 0]
    out[:, 1, 0::2, 1::2] = g[:, 0]
    out[:, 1, 1::2, 0::2] = g[:, 1]
    out[:, 1, 1::2, 1::2] = m[:, 3]
    out[:, 2, 0::2, 0::2] = cp[:, 3]
    out[:, 2, 0::2, 1::2] = cp[:, 4]
    out[:, 2, 1::2, 0::2] = m[:, 2]
    out[:, 2, 1::2, 1::2] = cp[:, 5]
    return out


# column offsets inside the packed [128, _WCOLS] bf16 stationary tensor.
# Every stationary is padded to 128 columns so the compiler's Fast Weight
# Load kicks in (NumWeights==128), hiding LDWEIGHTS behind the matmuls.
_WOFF = {"wf0": 0, "ww0": 128,
         "wse16": 256, "wsep": 384, "wch_rt": 512, "wch_g": 640}
_WCOLS = 768
# fp8 stationary tensor [128, 16, 96]: blocks 2i,2i+1 = conv i ky0/ky1 (DoubleRow
# pair), blocks 8+2i,8+2i+1 = conv i (ky2, zeros) DR pair; conv order
# (wf1, ww1, wf2, ww2); scaled by WSCALE
_W8ORDER = ("wf1", "ww1", "wf2", "ww2")


def pack_stationaries(st):
    wp = np.zeros((128, _WCOLS), np.float32)
    wp[:, 0:96] = st["wf0"]
    wp[:, 128:224] = st["ww0"]
    wp[0:96, 256:272] = st["wse16"]
    wp[0:96, 384:400] = st["wsep"]
    wp[:, 512:560] = st["wch_rt"]
    wp[0:60, 640:688] = st["wch_g"]
    w8 = np.zeros((128, 16, 96), np.float32)
    for i, nm in enumerate(_W8ORDER):
        w8[0:120, 2 * i, :] = st[nm][0] * WSCALE
        w8[0:120, 2 * i + 1, :] = st[nm][1] * WSCALE
        w8[0:120, 8 + 2 * i, :] = st[nm][2] * WSCALE
    return wp, w8


def build_program():
    from contextlib import ExitStack

    nc = bacc.Bacc(
        "TRN2", target_bir_lowering=False, debug=False, num_devices=N_CORES
    )
    r0 = nc.declare_dram_parameter("r0", [B_PC, 128, H, NW], BF16, isOutput=False)
    wpack = nc.declare_dram_parameter("wpack", [128, _WCOLS], BF16, isOutput=False)
    wpack8 = nc.declare_dram_parameter("wpack8", [128, 16, 96], F8, isOutput=False)
    out_cp = nc.declare_dram_parameter("out_cp", [B_PC, 48, H, NW], BF16, isOutput=True)
    out_g = nc.declare_dram_parameter("out_g", [B_PC, 16, H, NW], BF16, isOutput=True)

    Relu = mybir.ActivationFunctionType.Relu
    Exp = mybir.ActivationFunctionType.Exp
    Copy = mybir.ActivationFunctionType.Copy

    with tile.TileContext(nc) as tc, ExitStack() as ctx:
        const = ctx.enter_context(tc.tile_pool(name="const", bufs=1))
        r0pool = ctx.enter_context(tc.tile_pool(name="r0pool", bufs=11))
        p_r1 = ctx.enter_context(tc.tile_pool(name="r1", bufs=5))
        p_r2 = ctx.enter_context(tc.tile_pool(name="r2", bufs=6))
        p_grb = ctx.enter_context(tc.tile_pool(name="grb", bufs=7))
        p_gsh = ctx.enter_context(tc.tile_pool(name="gsh", bufs=4))
        p_act = ctx.enter_context(tc.tile_pool(name="acts", bufs=4))
        p_rcp = ctx.enter_context(tc.tile_pool(name="rcp", bufs=4))
        p_stg = ctx.enter_context(tc.tile_pool(name="stg", bufs=3))
        ps_mm = ctx.enter_context(tc.tile_pool(name="psmm", bufs=3, space="PSUM"))
        ps_sm = ctx.enter_context(tc.tile_pool(name="pssm", bufs=2, space="PSUM"))

        WC = const.tile([128, _WCOLS], BF16, tag="wpack_sb", name="wpack_sb")
        nc.sync.dma_start(out=WC[:], in_=wpack[:])
        WC8 = const.tile([128, 16, 96], F8, tag="wpack8_sb", name="wpack8_sb")
        nc.sync.dma_start(out=WC8[:], in_=wpack8[:])
        sb = {
            "wf0": WC[:, 0:128],
            "ww0": WC[:, 128:256],
            "wse16": WC[0:96, 256:384],
            "wsep": WC[0:96, 384:512],
            "wch_rt": WC[:, 512:640],
            "wch_g": WC[0:60, 640:768],
        }

        def w8dr(i):
            return WC8[0:120, 2 * i : 2 * i + 2, :]

        def w8ky2(i):
            return WC8[0:120, 8 + 2 * i : 8 + 2 * i + 2, :]

        def dr_moving(t, br, s, ky0=0, jstep=NW):
            # [120, 2, 8, NW] view of rows 8s+ky0..: j dim strides jstep elems
            mv = t[:, br : br + 1, 8 * s + ky0 : 8 * s + ky0 + 8, :]
            apv = mv.ap
            apv[1] = (jstep, 2)
            mv.ap = apv
            return mv

        r0s, r1, r2, grb, gshs, stash = {}, {}, {}, {}, {}, {}

        def load_r0(g):
            if g in r0s or not (0 <= g < NGROUP):
                return r0s.get(g)
            img, gi = divmod(g, NG_IMG)
            y0 = gi * GROWS
            rt = r0pool.tile([128, GROWS + 2, NW], BF16, name="rt")
            r0s[g] = rt
            if gi == 0:
                nc.vector.memset(rt[:, 0:1, :], 0.0)
                nc.sync.dma_start(
                    out=rt[:, 1 : GROWS + 2, :], in_=r0[img, :, 0 : GROWS + 1, :]
                )
            elif gi == NG_IMG - 1:
                nc.vector.memset(rt[:, GROWS + 1 : GROWS + 2, :], 0.0)
                nc.sync.dma_start(
                    out=rt[:, 0 : GROWS + 1, :],
                    in_=r0[img, :, y0 - 1 : y0 + GROWS, :],
                )
            else:
                nc.sync.dma_start(
                    out=rt[:], in_=r0[img, :, y0 - 1 : y0 + GROWS + 1, :]
                )
            return rt

        def get_rbuf(pool, dct, g):
            if g in dct or not (0 <= g < NGROUP):
                return dct.get(g)
            t = pool.tile([120, 2, GROWS + 2, NW], F8)
            dct[g] = t
            gi = g % NG_IMG
            if gi == 0:
                nc.gpsimd.memset(t[0:96, :, 0:1, :], 0.0)
            if gi == NG_IMG - 1:
                nc.gpsimd.memset(t[0:96, :, GROWS + 1 : GROWS + 2, :], 0.0)
            nc.gpsimd.memset(t[96:120, :, :, 0:1], 0.0)
            nc.gpsimd.memset(t[96:120, :, :, NW - 1 : NW], 0.0)
            return t

        def get_grb(g):
            if g in grb or not (0 <= g < NGROUP):
                return grb.get(g)
            t = p_grb.tile([20, GROWS + 2, NW], BF16, name="g")
            grb[g] = t
            gi = g % NG_IMG
            if gi == 0:
                nc.gpsimd.memset(t[:, 0:1, :], 0.0)
            if gi == NG_IMG - 1:
                nc.gpsimd.memset(t[:, GROWS + 1 : GROWS + 2, :], 0.0)
            nc.gpsimd.memset(t[:, :, 0:1], 0.0)
            nc.gpsimd.memset(t[:, :, NW - 1 : NW], 0.0)
            return t

        Max = mybir.AluOpType.max
        Mult = mybir.AluOpType.mult

        def evict_pair(ps, dct, br, g, gi, t, scale=1.0):
            # ps holds slabs 2t and 2t+1 in one [128, 16, NW] double tile
            nc.scalar.activation(
                out=dct[g][0:96, br, 16 * t + 1 : 16 * t + 17, :],
                in_=ps[0:96],
                func=Relu,
                scale=scale,
            )
            # boundary-row halo copies into neighbor tiles go to DVE
            if t == 0 and gi > 0:
                nc.vector.tensor_scalar(
                    out=dct[g - 1][0:96, br, GROWS + 1 : GROWS + 2, :],
                    in0=ps[0:96, 0:1, :],
                    scalar1=0.0,
                    scalar2=scale,
                    op0=Max,
                    op1=Mult,
                )
            if t == 1 and gi < NG_IMG - 1:
                nc.vector.tensor_scalar(
                    out=dct[g + 1][0:96, br, 0:1, :],
                    in0=ps[0:96, 15:16, :],
                    scalar1=0.0,
                    scalar2=scale,
                    op0=Max,
                    op1=Mult,
                )

        def strips(t):
            nc.gpsimd.dma_start(
                out=t[96:108, :, :, 1:NW], in_=t[84:96, :, :, 0 : NW - 1]
            )
            nc.gpsimd.dma_start(
                out=t[108:120, :, :, 0 : NW - 1], in_=t[0:12, :, :, 1:NW]
            )

        for T in range(NGROUP + 8):
            load_r0(T)
            load_r0(T + 1)
            load_r0(T + 2)
            # ---- stage A: group a = T; layer-0 convs ----
            a = T
            if 0 <= a < NGROUP:
                img, gi = divmod(a, NG_IMG)
                rt = r0s[a]
                get_rbuf(p_r1, r1, a)
                get_rbuf(p_r1, r1, a + 1)
                for br, nm in ((0, "wf0"), (1, "ww0")):
                    for t in range(2):
                        ps = ps_mm.tile([128, 16, NW], F32, tag="mm96", name="ps0")
                        for u in range(2):
                            s = 2 * t + u
                            nc.tensor.matmul(
                                ps[:, 8 * u : 8 * u + 8, :],
                                sb[nm],
                                rt[:, 8 * s + 1 : 8 * s + 9, :],
                                start=True,
                                stop=True,
                            )
                        evict_pair(ps, r1, br, a, gi, t)
            if 0 <= T - 1 < NGROUP:
                strips(r1[T - 1])

            # ---- stage B: group b = T - 2; layer-1 convs ----
            b = T - 2
            if 0 <= b < NGROUP:
                img, gi = divmod(b, NG_IMG)
                get_rbuf(p_r2, r2, b)
                get_rbuf(p_r2, r2, b + 1)
                for br, wi in ((0, 0), (1, 1)):
                    pss = [
                        ps_mm.tile([128, 16, NW], F32, tag="mm96", name="ps1")
                        for _ in range(2)
                    ]
                    for s in range(GS):
                        nc.tensor.matmul(
                            pss[s // 2][0:96, 8 * (s % 2) : 8 * (s % 2) + 8, :],
                            w8dr(wi),
                            dr_moving(r1[b], br, s),
                            start=True,
                            stop=False,
                            perf_mode=mybir.MatmulPerfMode.DoubleRow,
                        )
                    for s in range(GS):
                        nc.tensor.matmul(
                            pss[s // 2][0:96, 8 * (s % 2) : 8 * (s % 2) + 8, :],
                            w8ky2(wi),
                            dr_moving(r1[b], br, s, ky0=2, jstep=0),
                            start=False,
                            stop=True,
                            perf_mode=mybir.MatmulPerfMode.DoubleRow,
                        )
                    for t in range(2):
                        evict_pair(pss[t], r2, br, b, gi, t, scale=1.0 / WSCALE)
            if 0 <= T - 3 < NGROUP:
                strips(r2[T - 3])

            # ---- stage C: group c = T - 4; layer-2 convs + softmax sums ----
            c = T - 4
            if 0 <= c < NGROUP:
                img, gi = divmod(c, NG_IMG)
                gt = get_grb(c)
                get_grb(c + 1)
                Es, EPs = [], []
                pss = [
                    ps_mm.tile([128, 16, NW], F32, tag="mm96", name="psw2")
                    for _ in range(2)
                ]
                for s in range(GS):
                    nc.tensor.matmul(
                        pss[s // 2][0:96, 8 * (s % 2) : 8 * (s % 2) + 8, :],
                        w8dr(3),
                        dr_moving(r2[c], 1, s),
                        start=True,
                        stop=False,
                        perf_mode=mybir.MatmulPerfMode.DoubleRow,
                    )
                for s in range(GS):
                    nc.tensor.matmul(
                        pss[s // 2][0:96, 8 * (s % 2) : 8 * (s % 2) + 8, :],
                        w8ky2(3),
                        dr_moving(r2[c], 1, s, ky0=2, jstep=0),
                        start=False,
                        stop=True,
                        perf_mode=mybir.MatmulPerfMode.DoubleRow,
                    )
                for t in range(2):
                    E0 = p_act.tile([96, 16, NW], BF16, tag="E0", name="E0")
                    nc.scalar.activation(
                        out=E0[:], in_=pss[t][0:96], func=Exp, scale=1.0 / WSCALE
                    )
                    E = p_act.tile([96, 16, NW], BF16, tag="E", name="E")
                    nc.vector.tensor_scalar_max(E[:], E0[:], 1.0)
                    Es.append(E)
                pss = [
                    ps_mm.tile([128, 16, NW], F32, tag="mm96", name="psf2")
                    for _ in range(2)
                ]
                for s in range(GS):
                    nc.tensor.matmul(
                        pss[s // 2][0:96, 8 * (s % 2) : 8 * (s % 2) + 8, :],
                        w8dr(2),
                        dr_moving(r2[c], 0, s),
                        start=True,
                        stop=False,
                        perf_mode=mybir.MatmulPerfMode.DoubleRow,
                    )
                for s in range(GS):
                    nc.tensor.matmul(
                        pss[s // 2][0:96, 8 * (s % 2) : 8 * (s % 2) + 8, :],
                        w8ky2(2),
                        dr_moving(r2[c], 0, s, ky0=2, jstep=0),
                        start=False,
                        stop=True,
                        perf_mode=mybir.MatmulPerfMode.DoubleRow,
                    )
                for t in range(2):
                    EP = p_act.tile([96, 16, NW], BF16, tag="EP", name="EP")
                    nc.vector.scalar_tensor_tensor(
                        out=EP[:],
                        in0=pss[t][0:96],
                        scalar=0.0,
                        in1=Es[t][:],
                        op0=mybir.AluOpType.max,
                        op1=mybir.AluOpType.mult,
                    )
                    EPs.append(EP)
                rcps = []
                for s in range(GS):
                    pse = ps_sm.tile([128, 8, NW], F32, tag="sm", name="pse")
                    nc.tensor.matmul(
                        pse[:],
                        sb["wse16"],
                        Es[s // 2][:, 8 * (s % 2) : 8 * (s % 2) + 8, :],
                        start=True,
                        stop=True,
                    )
                    rcp = p_rcp.tile([16, 8, NW], F32, tag="rcp", name="rcp")
                    nc.vector.reciprocal_approx_fast(out=rcp[:], in_=pse[0:16])
                    rcps.append(rcp)
            # ---- stage D: group gD = T - 7; chroma conv + staging ----
            gD = T - 7
            if 0 <= gD < NGROUP:
                imgD, giD = divmod(gD, NG_IMG)
                gtD = grb[gD]
                rtD = r0s[gD]
                gshD = gshs[gD]
                stgt = p_stg.tile([48, GROWS, NW], BF16, name="stg")
                for t in range(2):
                    pc = ps_mm.tile([128, 16, NW], F32, tag="mm96", name="pc")
                    # chroma = conv(rb) + green_add m-parts (from r0)
                    #        - conv(g) + green_add g-parts (from gsh);
                    # all rt matmuls first, then all g, to halve weight swaps
                    for u in range(2):
                        s = 2 * t + u
                        nc.tensor.matmul(
                            pc[:, 8 * u : 8 * u + 8, :],
                            sb["wch_rt"],
                            rtD[:, 8 * s + 1 : 8 * s + 9, :],
                            start=True,
                            stop=False,
                        )
                    for u in range(2):
                        s = 2 * t + u
                        nc.tensor.matmul(
                            pc[:, 8 * u : 8 * u + 8, :],
                            sb["wch_g"],
                            gshD[:, 8 * s : 8 * s + 8, :],
                            start=False,
                            stop=True,
                        )
                    if t == 0:
                        nc.scalar.activation(
                            out=stgt[:, 0:16, :], in_=pc[0:48, :, :], func=Copy
                        )
                    else:
                        nc.vector.tensor_copy(
                            out=stgt[:, 16:32, :], in_=pc[0:48, :, :]
                        )
                stash[gD] = (imgD, giD * GROWS, stgt, gtD)


            # ---- stage C part 2: psep sums + green multiply ----
            if 0 <= c < NGROUP:
                for s in range(GS):
                    psep = ps_sm.tile([128, 8, NW], F32, tag="sm", name="psep")
                    nc.tensor.matmul(
                        psep[:],
                        sb["wsep"],
                        EPs[s // 2][:, 8 * (s % 2) : 8 * (s % 2) + 8, :],
                        start=True,
                        stop=True,
                    )
                    nc.vector.tensor_mul(
                        gt[0:16, 8 * s + 1 : 8 * s + 9, :], psep[0:16], rcps[s][:]
                    )
                    if s == 0 and gi > 0:
                        nc.vector.tensor_mul(
                            grb[c - 1][0:16, GROWS + 1 : GROWS + 2, :],
                            psep[0:16, 0:1, :],
                            rcps[s][:, 0:1, :],
                        )
                    if s == GS - 1 and gi < NG_IMG - 1:
                        nc.vector.tensor_mul(
                            grb[c + 1][0:16, 0:1, :],
                            psep[0:16, 7:8, :],
                            rcps[s][:, 7:8, :],
                        )

            # ---- D-prep: group p = T - 5; g x-strips + 3 ky-shifted g copies ----
            p = T - 5
            if 0 <= p < NGROUP:
                gt = grb[p]
                nc.sync.dma_start(out=gt[16:18, :, 1:NW], in_=gt[14:16, :, 0 : NW - 1])
                nc.sync.dma_start(out=gt[18:20, :, 0 : NW - 1], in_=gt[0:2, :, 1:NW])
                gs = p_gsh.tile([60, GROWS, NW], BF16, name="gsh")
                gshs[p] = gs
                for k in range(3):
                    nc.gpsimd.dma_start(
                        out=gs[20 * k : 20 * (k + 1), :, :], in_=gt[:, k : k + GROWS, :]
                    )

            if 0 <= T - 8 < NGROUP:
                imgD, y0D, stgt, gtD = stash.pop(T - 8)
                nc.sync.dma_start(
                    out=out_cp[imgD, :, y0D : y0D + GROWS, :], in_=stgt[:]
                )
                nc.sync.dma_start(
                    out=out_g[imgD, :, y0D : y0D + GROWS, :],
                    in_=gtD[0:16, 1 : GROWS + 1, :],
                )
                r0s.pop(T - 7, None)
                r1.pop(T - 3, None)
                r2.pop(T - 5, None)
                grb.pop(T - 8, None)
                gshs.pop(T - 8, None)

    nc.compile()
    return nc


_CACHE = {}


def kernel(mosaic, fw0, fw1, fw2, ww0, ww1, ww2, cw0, _trace=False):
    mosaic = np.asarray(mosaic, np.float32)
    r0_all = build_r0(mosaic)

    stat = {
        "wf0": build_w_l0(np.asarray(fw0, np.float32)),
        "ww0": build_w_l0(np.asarray(ww0, np.float32)),
        "wf1": build_w_int(np.asarray(fw1, np.float32)),
        "wf2": build_w_int(np.asarray(fw2, np.float32)),
        "ww1": build_w_int(np.asarray(ww1, np.float32)),
        "ww2": build_w_int(np.asarray(ww2, np.float32)),
    }
    stat["wse16"], stat["wsep"] = build_w_sums16()
    cw0f = np.asarray(cw0, np.float32)
    stat["wch_rt"] = build_w_chroma_rt(cw0f)
    stat["wch_g"] = build_w_chroma_g(cw0f)
    wp, w8 = pack_stationaries(stat)
    wpack = wp.astype(BF16NP)
    wpack8 = w8.astype(F8NP)

    if "nc" not in _CACHE:
        _CACHE["nc"] = build_program()
    nc = _CACHE["nc"]

    in_maps = []
    for c in range(N_CORES):
        in_maps.append(
            {"r0": np.ascontiguousarray(r0_all[c * B_PC : (c + 1) * B_PC]),
             "wpack": wpack, "wpack8": wpack8}
        )

    res = run_bass_kernel_spmd(nc, in_maps, list(range(N_CORES)), trace=_trace)
    outs = []
    for c in range(N_CORES):
        outs.append(
            assemble_output(
                mosaic[c * B_PC : (c + 1) * B_PC],
                res.results[c]["out_cp"],
                res.results[c]["out_g"],
            )
        )
    full = np.concatenate(outs, axis=0)
    if _trace:
        return full, res
    return full

